# revision 11
# baseline (speedup 1.0000x reference)
"""Full-width attention (B=4, S=2048, D=1024, no head split) on 8 TRN2 cores.

Sharding: data-parallel over (batch, query-half) -> 8 shards. Core c handles
batch b = c//2, query rows [h*1024, (h+1)*1024) with h = c%2.

v2 rewrite vs the K-side-folding baseline:
  - Fold Wq/Wk into the QUERY side: Q'' = (x_q M + w3) / 8 with M = Wq^T Wk
    and w3 = Wk^T bq. Then scores^T[k,q] = sum_e x[k,e] Q''[q,e] needs NO key
    projection at all -- the redundant per-pair M x^T (128 MMs) and the t3
    bias matmuls (32 MMs) disappear. The per-key softmax bias folds into Q''
    as a per-partition bias on the projection evacuation (free on Act).
  - All big matmuls run bf16 x bf16: same 1 cycle/row streaming as f32r but
    the 128-col weight load uses FWL (2x) and hides under the reorder
    window, cutting the per-MM gap. Also halves SBUF/DMA so x, x^T, E, Wv
    are all SBUF-resident -- zero DMA in the steady state.
  - Query half selection without a separate upload: the host rotates the
    KEY axis by h*1024 in both x^T (scores lhsT) and x (PX lhsT); attention
    is permutation-invariant over keys, and the core's queries are always
    columns 0:1024 of its rotated x^T.
  - Softmax without max-subtraction (|scores| <= ~25, exp safe in f32):
    E = exp(scores^T), rowsum via DVE accumulation + one ones-matmul
    partition-reduce, [1,1024] -> [128,8] recips via DRAM bounce.
  - out[q,e] = (x^T E)^T Wv^T scaled by 1/rowsum + bv (bv folded in after
    normalization since softmax rows sum to 1).
"""

import math
from contextlib import ExitStack

import numpy as np

P = 128
B, S, D = 4, 2048, 1024
SQ = 1024  # query rows per core
KO8 = 8  # 1024 contraction / 128
KO16 = 16  # 2048 contraction / 128
N_CORES = 8


def build_bass():
    from concourse import bacc
    import concourse.mybir as mybir
    from concourse.tile import TileContext

    f32 = mybir.dt.float32
    f32r = mybir.dt.float32r
    bf16 = mybir.dt.bfloat16
    AF = mybir.ActivationFunctionType

    nc = bacc.Bacc(
        "TRN2",
        target_bir_lowering=False,
        debug=False,
        enable_asserts=False,
        num_devices=N_CORES,
    )

    xT = nc.dram_tensor("xT", [D, S], bf16, kind="ExternalInput")
    xn = nc.dram_tensor("xn", [S, D], bf16, kind="ExternalInput")
    mT = nc.dram_tensor("mT", [D, D], bf16, kind="ExternalInput")
    wvT = nc.dram_tensor("wvT", [D, D], bf16, kind="ExternalInput")
    w3 = nc.dram_tensor("w3", [P, KO8], f32, kind="ExternalInput")
    bvb = nc.dram_tensor("bvb", [P, D], f32, kind="ExternalInput")
    out = nc.dram_tensor("out", [SQ, D], f32, kind="ExternalOutput")

    xT_r = xT[:, :].rearrange("(ko p) s -> p ko s", p=P)
    xn_r = xn[:, :].rearrange("(ko p) d -> p ko d", p=P)
    mT_r = mT[:, :].rearrange("(ko p) e -> p ko e", p=P)
    wvT_r = wvT[:, :].rearrange("(ko p) e -> p ko e", p=P)

    with TileContext(nc) as tc, ExitStack() as ctx:
        cst_p = ctx.enter_context(tc.tile_pool(name="cst", bufs=1))
        big_p = ctx.enter_context(tc.tile_pool(name="big", bufs=1))
        out_p = ctx.enter_context(tc.tile_pool(name="osp", bufs=3))
        psA_p = ctx.enter_context(tc.tile_pool(name="psA", bufs=3, space="PSUM"))
        psB_p = ctx.enter_context(tc.tile_pool(name="psB", bufs=2, space="PSUM"))
        psC_p = ctx.enter_context(tc.tile_pool(name="psC", bufs=2, space="PSUM"))
        psR_p = ctx.enter_context(tc.tile_pool(name="psR", bufs=1, space="PSUM"))
        dram_p = ctx.enter_context(tc.tile_pool(name="drp", bufs=1, space="DRAM"))

        # warmup operand comes from a memset, not a DMA, so the PE can start
        # ramping the instant the engines clear the startup barrier (walrus
        # rejects memset on f32r tiles, so memset f32 and bitcast for the PE)
        ones_f = cst_p.tile([P, 512], f32, tag="ones", name="ones_f")
        nc.vector.memset(ones_f[:], 1.0)
        ones_t = ones_f[:, :].bitcast(f32r)
        w3_t = cst_p.tile([P, KO8], f32, tag="w3", name="w3_t")
        nc.gpsimd.dma_start(w3_t[:], w3[:, :])
        bvb_t = cst_p.tile([P, D], f32, tag="bvb", name="bvb_t")
        nc.gpsimd.dma_start(bvb_t[:], bvb[:, :])

        # big residents
        xt_sb = big_p.tile([P, KO8, S], bf16, tag="xt", name="xt_sb")
        xn_sb = big_p.tile([P, KO16, D], bf16, tag="xn", name="xn_sb")
        m_sb = big_p.tile([P, KO8, D], bf16, tag="m", name="m_sb")
        wv_sb = big_p.tile([P, KO8, D], bf16, tag="wv", name="wv_sb")
        qt_sb = big_p.tile([P, KO8, SQ], bf16, tag="qt", name="qt_sb")
        e_sb = [
            big_p.tile([P, KO16, 512], bf16, tag=f"E{qc}", name=f"e_sb{qc}")
            for qc in range(2)
        ]
        px_sb = big_p.tile([P, KO8, SQ], bf16, tag="px", name="px_sb")
        racc = [
            cst_p.tile([P, 512], f32r, tag=f"racc{qc}", name=f"racc{qc}")
            for qc in range(2)
        ]
        rs_dram = dram_p.tile([1, SQ], f32, tag="rsd", name="rs_dram")

        # Phase-1 feed: per-ko full-width chunks (2KB/partition contiguous —
        # finer column splits halve DMA efficiency), m on the sync ring and
        # x^T query columns on the scalar ring in parallel. The first Q''
        # group then starts as soon as chunk ko=0 of both lands (~12.5us)
        # and pipelines with the remaining arrivals. Everything else is
        # emitted BEHIND phase-1 evacuations on the scalar queue so its
        # transfers cannot steal HBM bandwidth from the critical path.
        # both ko=0 chunks go first on the sync ring — it arms ~1.8us before
        # the scalar ring, and chunk 0's completion semaphore is the gate
        # for the whole phase-1 pipeline
        nc.sync.dma_start(m_sb[:, 0, :], mT_r[:, 0, :])
        nc.sync.dma_start(xt_sb[:, 0, 0:SQ], xT_r[:, 0, 0:SQ])
        for ko in range(1, KO8):
            eng = nc.sync if ko % 2 == 0 else nc.scalar
            eng.dma_start(m_sb[:, ko, :], mT_r[:, ko, :])
            eng2 = nc.scalar if ko % 2 == 0 else nc.sync
            eng2.dma_start(xt_sb[:, ko, 0:SQ], xT_r[:, ko, 0:SQ])

        # PE warm-up on the ones tile: keeps the HAM activity window busy so
        # real matmuls run at 2.4 GHz, and covers the phase-1 DMA latency.
        warm = psR_p.tile([1, 512], f32, tag="psR", name="warm")
        for _ in range(16):
            nc.tensor.matmul(warm[:], ones_t[:, 0:1], ones_t[:, :])

        # ---- Phase 1: Q''T[e, q] = M^T x_q^T + w3 (scaled by 1/8 on host) --
        # ko-OUTER with all 8 eo accumulations held open across the full
        # PSUM bank set: each ko step needs only chunk ko of m/x^T, so the
        # whole sweep paces with the DMA chunk arrivals instead of
        # head-of-line blocking on the first eo group's last chunk.
        def q_sweep(qc):
            banks = [
                psA_p.tile([P, 512], f32, tag="psA", name=f"qp{qc}a{i}")
                for i in range(3)
            ] + [
                psB_p.tile([P, 512], f32, tag="psB", name=f"qp{qc}b{i}")
                for i in range(2)
            ] + [
                psC_p.tile([P, 512], f32, tag="psC", name=f"qp{qc}c{i}")
                for i in range(2)
            ] + [psR_p.tile([P, 512], f32, tag="psR", name=f"qp{qc}r")]
            for ko in range(KO8):
                for eo in range(KO8):
                    nc.tensor.matmul(
                        banks[eo][:],
                        m_sb[:, ko, eo * P : (eo + 1) * P],
                        xt_sb[:, ko, qc * 512 : (qc + 1) * 512],
                        start=(ko == 0), stop=(ko == KO8 - 1),
                    )
            for eo in range(KO8):
                nc.scalar.activation(
                    qt_sb[:, eo, qc * 512 : (qc + 1) * 512],
                    banks[eo][:], AF.Identity, bias=w3_t[:, eo : eo + 1],
                )

        q_sweep(0)
        # non-critical loads issue behind the first evacuations so their
        # transfers cannot steal HBM bandwidth from the phase-1 chunks
        for kp in range(2):
            nc.scalar.dma_start(
                xt_sb[:, 4 * kp : 4 * kp + 4, SQ:S],
                xT_r[:, 4 * kp : 4 * kp + 4, SQ:S],
            )
        nc.scalar.dma_start(wv_sb[:, :, :], wvT_r[:, :, :])
        q_sweep(1)
        for kp in range(2):
            nc.scalar.dma_start(
                xn_sb[:, 8 * kp : 8 * kp + 8, :], xn_r[:, 8 * kp : 8 * kp + 8, :]
            )

        # ---- Phase 2: scores^T -> exp -> E (bf16), rowsum acc on DVE ------
        for kidx in range(KO16):
            pa = psA_p.tile([P, 512], f32, tag="psA", name="spa")
            pb = psB_p.tile([P, 512], f32, tag="psB", name="spb")
            for eo in range(KO8):
                lh = xt_sb[:, eo, kidx * P : (kidx + 1) * P]
                nc.tensor.matmul(
                    pa[:], lh, qt_sb[:, eo, 0:512],
                    start=(eo == 0), stop=(eo == KO8 - 1),
                )
                nc.tensor.matmul(
                    pb[:], lh, qt_sb[:, eo, 512:1024],
                    start=(eo == 0), stop=(eo == KO8 - 1),
                )
            nc.scalar.activation(e_sb[0][:, kidx, :], pa[:], AF.Exp)
            nc.scalar.activation(e_sb[1][:, kidx, :], pb[:], AF.Exp)
            for qc in range(2):
                if kidx == 0:
                    nc.vector.tensor_copy(racc[qc][:], e_sb[qc][:, 0, :])
                else:
                    nc.vector.tensor_add(
                        racc[qc][:], racc[qc][:], e_sb[qc][:, kidx, :]
                    )

        # ---- Phase 3: PX^T[d, q] = sum_k x[k, d] E[k, q] -------------------
        for dc in range(KO8):
            pp = psA_p.tile([P, 512], f32, tag="psA", name="ppx")
            for ko in range(KO16):
                nc.tensor.matmul(
                    pp[:],
                    xn_sb[:, ko, dc * P : (dc + 1) * P],
                    e_sb[0][:, ko, :],
                    start=(ko == 0), stop=(ko == KO16 - 1),
                )
            nc.scalar.copy(px_sb[:, dc, 0:512], pp[:])

        # rowsum partition-reduce + [1,1024] -> [128,8] recip via DRAM bounce
        # (PE cost ~2 tiny matmuls; bounce hides under PX)
        for qc in range(2):
            pr = psR_p.tile([1, 512], f32, tag="psR", name="pr")
            nc.tensor.matmul(pr[:], ones_t[:, 0:1], racc[qc][:])
            rrow = cst_p.tile([1, 512], f32, tag=f"rr{qc}", name=f"rrow{qc}")
            nc.scalar.copy(rrow[:], pr[:])
            nc.sync.dma_start(rs_dram[0:1, qc * 512 : (qc + 1) * 512], rrow[:])
        rsum_t = cst_p.tile([P, 8], f32, tag="rst", name="rsum_t")
        nc.sync.dma_start(rsum_t[:, :], rs_dram[0, :].rearrange("(g p) -> p g", p=P))
        recip = cst_p.tile([P, 8], f32, tag="recip", name="recip")
        nc.vector.reciprocal(recip[:], rsum_t[:])

        for dc in range(KO8):
            pp = psA_p.tile([P, 512], f32, tag="psA", name="ppx")
            for ko in range(KO16):
                nc.tensor.matmul(
                    pp[:],
                    xn_sb[:, ko, dc * P : (dc + 1) * P],
                    e_sb[1][:, ko, :],
                    start=(ko == 0), stop=(ko == KO16 - 1),
                )
            nc.scalar.copy(px_sb[:, dc, 512:1024], pp[:])

        # ---- Phase 4: out[q, e] = PX^T.T Wv^T / rowsum + bv ---------------
        for g in range(8):
            pb = psB_p.tile([P, 512], f32, tag="psB", name="avb")
            pc = psC_p.tile([P, 512], f32, tag="psC", name="avc")
            for dc in range(KO8):
                lh = px_sb[:, dc, g * P : (g + 1) * P]
                nc.tensor.matmul(
                    pb[:], lh, wv_sb[:, dc, 0:512],
                    start=(dc == 0), stop=(dc == KO8 - 1),
                )
                nc.tensor.matmul(
                    pc[:], lh, wv_sb[:, dc, 512:1024],
                    start=(dc == 0), stop=(dc == KO8 - 1),
                )
            # fused (psum * recip) + bv straight from PSUM on DVE; each half
            # DMAs out as soon as its fuse lands so the tail never waits on
            # a combined transfer
            o = out_p.tile([P, D], f32, tag="ost", name="ost")
            mul, add = mybir.AluOpType.mult, mybir.AluOpType.add
            # (Pool/gpsimd cannot read PSUM on TRN2 — both halves on DVE)
            for half, ps, eng in ((0, pb, nc.vector), (1, pc, nc.vector)):
                eng.scalar_tensor_tensor(
                    o[:, half * 512 : (half + 1) * 512],
                    ps[:],
                    recip[:, g : g + 1],
                    bvb_t[:, half * 512 : (half + 1) * 512],
                    mul,
                    add,
                )
                nc.sync.dma_start(
                    out[g * P : (g + 1) * P, half * 512 : (half + 1) * 512],
                    o[:, half * 512 : (half + 1) * 512],
                )

    nc.finalize()
    return nc


def make_in_maps(x, Wq, bq, Wk, bk, Wv, bv):
    """Build the 8 per-core input maps from full inputs."""
    import ml_dtypes

    bf = ml_dtypes.bfloat16
    x = np.asarray(x, dtype=np.float32)
    inv8 = 1.0 / math.sqrt(D // 16)  # 1/sqrt(d_key=64) = 1/8
    # scores = x_q (Wq^T Wk) x_k^T / 8 + x_k.(Wk^T bq)/8 (+ softmax-invariant
    # per-query terms, dropped). Both folded into the query-side projection.
    M8 = (
        (np.asarray(Wq, np.float64).T @ np.asarray(Wk, np.float64)) * inv8
    ).astype(bf)
    w3 = (
        (np.asarray(Wk, np.float64).T @ np.asarray(bq, np.float64)) * inv8
    ).astype(np.float32)
    w3_np = np.ascontiguousarray(w3.reshape(KO8, P).T)
    wvT = np.ascontiguousarray(np.asarray(Wv, np.float32).T.astype(bf))
    bvb = np.ascontiguousarray(
        np.broadcast_to(np.asarray(bv, np.float32), (P, D))
    )
    in_maps = []
    for c in range(N_CORES):
        b, h = c // 2, c % 2
        # rotate the key axis by h*SQ so this core's queries are always
        # columns 0:SQ of xT; attention is permutation-invariant over keys
        # as long as xT (scores lhsT) and xn (PX lhsT) rotate together.
        xb = np.roll(x[b], -h * SQ, axis=0)
        in_maps.append(
            {
                "xT": np.ascontiguousarray(xb.T.astype(bf)),
                "xn": np.ascontiguousarray(xb.astype(bf)),
                "mT": M8,
                "wvT": wvT,
                "w3": w3_np,
                "bvb": bvb,
            }
        )
    return in_maps


_NC_CACHE = None


def get_nc():
    global _NC_CACHE
    if _NC_CACHE is None:
        _NC_CACHE = build_bass()
    return _NC_CACHE


def kernel(x, Wq, bq, Wk, bk, Wv, bv, **run_kwargs):
    from concourse.bass_utils import run_bass_kernel_spmd

    nc = get_nc()
    in_maps = make_in_maps(x, Wq, bq, Wk, bk, Wv, bv)
    res = run_bass_kernel_spmd(
        nc, in_maps, core_ids=list(range(N_CORES)), **run_kwargs
    )
    out = np.empty((B, S, D), dtype=np.float32)
    for c in range(N_CORES):
        b, h = c // 2, c % 2
        out[b, h * SQ : (h + 1) * SQ, :] = res.results[c]["out"]
    if run_kwargs.get("trace"):
        kernel.last_results = res
    return out


# revision 12
# speedup vs baseline: 1.0079x; 1.0079x over previous
"""Full-width attention (B=4, S=2048, D=1024, no head split) on 8 TRN2 cores.

Sharding: data-parallel over (batch, query-half) -> 8 shards. Core c handles
batch b = c//2, query rows [h*1024, (h+1)*1024) with h = c%2.

v2 rewrite vs the K-side-folding baseline:
  - Fold Wq/Wk into the QUERY side: Q'' = (x_q M + w3) / 8 with M = Wq^T Wk
    and w3 = Wk^T bq. Then scores^T[k,q] = sum_e x[k,e] Q''[q,e] needs NO key
    projection at all -- the redundant per-pair M x^T (128 MMs) and the t3
    bias matmuls (32 MMs) disappear. The per-key softmax bias folds into Q''
    as a per-partition bias on the projection evacuation (free on Act).
  - All big matmuls run bf16 x bf16: same 1 cycle/row streaming as f32r but
    the 128-col weight load uses FWL (2x) and hides under the reorder
    window, cutting the per-MM gap. Also halves SBUF/DMA so x, x^T, E, Wv
    are all SBUF-resident -- zero DMA in the steady state.
  - Query half selection without a separate upload: the host rotates the
    KEY axis by h*1024 in both x^T (scores lhsT) and x (PX lhsT); attention
    is permutation-invariant over keys, and the core's queries are always
    columns 0:1024 of its rotated x^T.
  - Softmax without max-subtraction (|scores| <= ~25, exp safe in f32):
    E = exp(scores^T), rowsum via DVE accumulation + one ones-matmul
    partition-reduce, [1,1024] -> [128,8] recips via DRAM bounce.
  - out[q,e] = (x^T E)^T Wv^T scaled by 1/rowsum + bv (bv folded in after
    normalization since softmax rows sum to 1).
"""

import math
from contextlib import ExitStack

import numpy as np

P = 128
B, S, D = 4, 2048, 1024
SQ = 1024  # query rows per core
KO8 = 8  # 1024 contraction / 128
KO16 = 16  # 2048 contraction / 128
N_CORES = 8


def build_bass():
    from concourse import bacc
    import concourse.mybir as mybir
    from concourse.tile import TileContext

    f32 = mybir.dt.float32
    f32r = mybir.dt.float32r
    bf16 = mybir.dt.bfloat16
    AF = mybir.ActivationFunctionType

    nc = bacc.Bacc(
        "TRN2",
        target_bir_lowering=False,
        debug=False,
        enable_asserts=False,
        num_devices=N_CORES,
    )

    xT = nc.dram_tensor("xT", [D, S], bf16, kind="ExternalInput")
    xn = nc.dram_tensor("xn", [S, D], bf16, kind="ExternalInput")
    mT = nc.dram_tensor("mT", [D, D], bf16, kind="ExternalInput")
    wvT = nc.dram_tensor("wvT", [D, D], bf16, kind="ExternalInput")
    w3 = nc.dram_tensor("w3", [P, KO8], f32, kind="ExternalInput")
    bvb = nc.dram_tensor("bvb", [P, D], f32, kind="ExternalInput")
    out = nc.dram_tensor("out", [SQ, D], f32, kind="ExternalOutput")

    xT_r = xT[:, :].rearrange("(ko p) s -> p ko s", p=P)
    xn_r = xn[:, :].rearrange("(ko p) d -> p ko d", p=P)
    mT_r = mT[:, :].rearrange("(ko p) e -> p ko e", p=P)
    wvT_r = wvT[:, :].rearrange("(ko p) e -> p ko e", p=P)

    with TileContext(nc) as tc, ExitStack() as ctx:
        cst_p = ctx.enter_context(tc.tile_pool(name="cst", bufs=1))
        big_p = ctx.enter_context(tc.tile_pool(name="big", bufs=1))
        out_p = ctx.enter_context(tc.tile_pool(name="osp", bufs=3))
        psA_p = ctx.enter_context(tc.tile_pool(name="psA", bufs=3, space="PSUM"))
        psB_p = ctx.enter_context(tc.tile_pool(name="psB", bufs=2, space="PSUM"))
        psC_p = ctx.enter_context(tc.tile_pool(name="psC", bufs=2, space="PSUM"))
        psR_p = ctx.enter_context(tc.tile_pool(name="psR", bufs=1, space="PSUM"))
        dram_p = ctx.enter_context(tc.tile_pool(name="drp", bufs=1, space="DRAM"))

        # warmup operand comes from a memset, not a DMA, so the PE can start
        # ramping the instant the engines clear the startup barrier (walrus
        # rejects memset on f32r tiles, so memset f32 and bitcast for the PE)
        ones_f = cst_p.tile([P, 512], f32, tag="ones", name="ones_f")
        nc.vector.memset(ones_f[:], 1.0)
        ones_t = ones_f[:, :].bitcast(f32r)
        w3_t = cst_p.tile([P, KO8], f32, tag="w3", name="w3_t")
        nc.gpsimd.dma_start(w3_t[:], w3[:, :])
        bvb_t = cst_p.tile([P, D], f32, tag="bvb", name="bvb_t")
        nc.gpsimd.dma_start(bvb_t[:], bvb[:, :])

        # big residents
        xt_sb = big_p.tile([P, KO8, S], bf16, tag="xt", name="xt_sb")
        xn_sb = big_p.tile([P, KO16, D], bf16, tag="xn", name="xn_sb")
        m_sb = big_p.tile([P, KO8, D], bf16, tag="m", name="m_sb")
        wv_sb = big_p.tile([P, KO8, D], bf16, tag="wv", name="wv_sb")
        qt_sb = big_p.tile([P, KO8, SQ], bf16, tag="qt", name="qt_sb")
        e_sb = [
            big_p.tile([P, KO16, 512], bf16, tag=f"E{qc}", name=f"e_sb{qc}")
            for qc in range(2)
        ]
        px_sb = big_p.tile([P, KO8, SQ], bf16, tag="px", name="px_sb")
        racc = [
            cst_p.tile([P, 512], f32r, tag=f"racc{qc}", name=f"racc{qc}")
            for qc in range(2)
        ]
        rs_dram = dram_p.tile([1, SQ], f32, tag="rsd", name="rs_dram")

        # Phase-1 feed: per-ko full-width chunks (2KB/partition contiguous —
        # finer column splits halve DMA efficiency), m on the sync ring and
        # x^T query columns on the scalar ring in parallel. The first Q''
        # group then starts as soon as chunk ko=0 of both lands (~12.5us)
        # and pipelines with the remaining arrivals. Everything else is
        # emitted BEHIND phase-1 evacuations on the scalar queue so its
        # transfers cannot steal HBM bandwidth from the critical path.
        for ko in range(KO8):
            nc.sync.dma_start(m_sb[:, ko, :], mT_r[:, ko, :])
            nc.scalar.dma_start(xt_sb[:, ko, 0:SQ], xT_r[:, ko, 0:SQ])

        # PE warm-up on the ones tile: keeps the HAM activity window busy so
        # real matmuls run at 2.4 GHz, and covers the phase-1 DMA latency.
        warm = psR_p.tile([1, 512], f32, tag="psR", name="warm")
        for _ in range(14):
            nc.tensor.matmul(warm[:], ones_t[:, 0:1], ones_t[:, :])

        # ---- Phase 1: Q''T[e, q] = M^T x_q^T + w3 (scaled by 1/8 on host) --
        # ko-OUTER with all 8 eo accumulations held open across the full
        # PSUM bank set: each ko step needs only chunk ko of m/x^T, so the
        # whole sweep paces with the DMA chunk arrivals instead of
        # head-of-line blocking on the first eo group's last chunk.
        def q_sweep(qc):
            banks = [
                psA_p.tile([P, 512], f32, tag="psA", name=f"qp{qc}a{i}")
                for i in range(3)
            ] + [
                psB_p.tile([P, 512], f32, tag="psB", name=f"qp{qc}b{i}")
                for i in range(2)
            ] + [
                psC_p.tile([P, 512], f32, tag="psC", name=f"qp{qc}c{i}")
                for i in range(2)
            ] + [psR_p.tile([P, 512], f32, tag="psR", name=f"qp{qc}r")]
            for ko in range(KO8):
                for eo in range(KO8):
                    nc.tensor.matmul(
                        banks[eo][:],
                        m_sb[:, ko, eo * P : (eo + 1) * P],
                        xt_sb[:, ko, qc * 512 : (qc + 1) * 512],
                        start=(ko == 0), stop=(ko == KO8 - 1),
                    )
            for eo in range(KO8):
                nc.scalar.activation(
                    qt_sb[:, eo, qc * 512 : (qc + 1) * 512],
                    banks[eo][:], AF.Identity, bias=w3_t[:, eo : eo + 1],
                )

        q_sweep(0)
        # non-critical loads issue behind the first evacuations so their
        # transfers cannot steal HBM bandwidth from the phase-1 chunks
        for kp in range(2):
            nc.scalar.dma_start(
                xt_sb[:, 4 * kp : 4 * kp + 4, SQ:S],
                xT_r[:, 4 * kp : 4 * kp + 4, SQ:S],
            )
        nc.scalar.dma_start(wv_sb[:, :, :], wvT_r[:, :, :])
        q_sweep(1)
        for kp in range(2):
            nc.scalar.dma_start(
                xn_sb[:, 8 * kp : 8 * kp + 8, :], xn_r[:, 8 * kp : 8 * kp + 8, :]
            )

        # ---- Phase 2: scores^T -> exp -> E (bf16), rowsum acc on DVE ------
        for kidx in range(KO16):
            pa = psA_p.tile([P, 512], f32, tag="psA", name="spa")
            pb = psB_p.tile([P, 512], f32, tag="psB", name="spb")
            for eo in range(KO8):
                lh = xt_sb[:, eo, kidx * P : (kidx + 1) * P]
                nc.tensor.matmul(
                    pa[:], lh, qt_sb[:, eo, 0:512],
                    start=(eo == 0), stop=(eo == KO8 - 1),
                )
                nc.tensor.matmul(
                    pb[:], lh, qt_sb[:, eo, 512:1024],
                    start=(eo == 0), stop=(eo == KO8 - 1),
                )
            nc.scalar.activation(e_sb[0][:, kidx, :], pa[:], AF.Exp)
            nc.scalar.activation(e_sb[1][:, kidx, :], pb[:], AF.Exp)
            for qc in range(2):
                if kidx == 0:
                    nc.vector.tensor_copy(racc[qc][:], e_sb[qc][:, 0, :])
                else:
                    nc.vector.tensor_add(
                        racc[qc][:], racc[qc][:], e_sb[qc][:, kidx, :]
                    )

        # ---- Phase 3: PX^T[d, q] = sum_k x[k, d] E[k, q] -------------------
        for dc in range(KO8):
            pp = psA_p.tile([P, 512], f32, tag="psA", name="ppx")
            for ko in range(KO16):
                nc.tensor.matmul(
                    pp[:],
                    xn_sb[:, ko, dc * P : (dc + 1) * P],
                    e_sb[0][:, ko, :],
                    start=(ko == 0), stop=(ko == KO16 - 1),
                )
            nc.scalar.copy(px_sb[:, dc, 0:512], pp[:])

        # rowsum partition-reduce + [1,1024] -> [128,8] recip via DRAM bounce
        # (PE cost ~2 tiny matmuls; bounce hides under PX)
        for qc in range(2):
            pr = psR_p.tile([1, 512], f32, tag="psR", name="pr")
            nc.tensor.matmul(pr[:], ones_t[:, 0:1], racc[qc][:])
            rrow = cst_p.tile([1, 512], f32, tag=f"rr{qc}", name=f"rrow{qc}")
            nc.scalar.copy(rrow[:], pr[:])
            nc.sync.dma_start(rs_dram[0:1, qc * 512 : (qc + 1) * 512], rrow[:])
        rsum_t = cst_p.tile([P, 8], f32, tag="rst", name="rsum_t")
        nc.sync.dma_start(rsum_t[:, :], rs_dram[0, :].rearrange("(g p) -> p g", p=P))
        recip = cst_p.tile([P, 8], f32, tag="recip", name="recip")
        nc.vector.reciprocal(recip[:], rsum_t[:])

        for dc in range(KO8):
            pp = psA_p.tile([P, 512], f32, tag="psA", name="ppx")
            for ko in range(KO16):
                nc.tensor.matmul(
                    pp[:],
                    xn_sb[:, ko, dc * P : (dc + 1) * P],
                    e_sb[1][:, ko, :],
                    start=(ko == 0), stop=(ko == KO16 - 1),
                )
            nc.scalar.copy(px_sb[:, dc, 512:1024], pp[:])

        # ---- Phase 4: out[q, e] = PX^T.T Wv^T / rowsum + bv ---------------
        for g in range(8):
            pb = psB_p.tile([P, 512], f32, tag="psB", name="avb")
            pc = psC_p.tile([P, 512], f32, tag="psC", name="avc")
            for dc in range(KO8):
                lh = px_sb[:, dc, g * P : (g + 1) * P]
                nc.tensor.matmul(
                    pb[:], lh, wv_sb[:, dc, 0:512],
                    start=(dc == 0), stop=(dc == KO8 - 1),
                )
                nc.tensor.matmul(
                    pc[:], lh, wv_sb[:, dc, 512:1024],
                    start=(dc == 0), stop=(dc == KO8 - 1),
                )
            # fused (psum * recip) + bv straight from PSUM on DVE; each half
            # DMAs out as soon as its fuse lands so the tail never waits on
            # a combined transfer
            o = out_p.tile([P, D], f32, tag="ost", name="ost")
            mul, add = mybir.AluOpType.mult, mybir.AluOpType.add
            # (Pool/gpsimd cannot read PSUM on TRN2 — both halves on DVE)
            for half, ps, eng in ((0, pb, nc.vector), (1, pc, nc.vector)):
                eng.scalar_tensor_tensor(
                    o[:, half * 512 : (half + 1) * 512],
                    ps[:],
                    recip[:, g : g + 1],
                    bvb_t[:, half * 512 : (half + 1) * 512],
                    mul,
                    add,
                )
                nc.sync.dma_start(
                    out[g * P : (g + 1) * P, half * 512 : (half + 1) * 512],
                    o[:, half * 512 : (half + 1) * 512],
                )

    nc.finalize()
    return nc


def make_in_maps(x, Wq, bq, Wk, bk, Wv, bv):
    """Build the 8 per-core input maps from full inputs."""
    import ml_dtypes

    bf = ml_dtypes.bfloat16
    x = np.asarray(x, dtype=np.float32)
    inv8 = 1.0 / math.sqrt(D // 16)  # 1/sqrt(d_key=64) = 1/8
    # scores = x_q (Wq^T Wk) x_k^T / 8 + x_k.(Wk^T bq)/8 (+ softmax-invariant
    # per-query terms, dropped). Both folded into the query-side projection.
    M8 = (
        (np.asarray(Wq, np.float64).T @ np.asarray(Wk, np.float64)) * inv8
    ).astype(bf)
    w3 = (
        (np.asarray(Wk, np.float64).T @ np.asarray(bq, np.float64)) * inv8
    ).astype(np.float32)
    w3_np = np.ascontiguousarray(w3.reshape(KO8, P).T)
    wvT = np.ascontiguousarray(np.asarray(Wv, np.float32).T.astype(bf))
    bvb = np.ascontiguousarray(
        np.broadcast_to(np.asarray(bv, np.float32), (P, D))
    )
    in_maps = []
    for c in range(N_CORES):
        b, h = c // 2, c % 2
        # rotate the key axis by h*SQ so this core's queries are always
        # columns 0:SQ of xT; attention is permutation-invariant over keys
        # as long as xT (scores lhsT) and xn (PX lhsT) rotate together.
        xb = np.roll(x[b], -h * SQ, axis=0)
        in_maps.append(
            {
                "xT": np.ascontiguousarray(xb.T.astype(bf)),
                "xn": np.ascontiguousarray(xb.astype(bf)),
                "mT": M8,
                "wvT": wvT,
                "w3": w3_np,
                "bvb": bvb,
            }
        )
    return in_maps


_NC_CACHE = None


def get_nc():
    global _NC_CACHE
    if _NC_CACHE is None:
        _NC_CACHE = build_bass()
    return _NC_CACHE


def kernel(x, Wq, bq, Wk, bk, Wv, bv, **run_kwargs):
    from concourse.bass_utils import run_bass_kernel_spmd

    nc = get_nc()
    in_maps = make_in_maps(x, Wq, bq, Wk, bk, Wv, bv)
    res = run_bass_kernel_spmd(
        nc, in_maps, core_ids=list(range(N_CORES)), **run_kwargs
    )
    out = np.empty((B, S, D), dtype=np.float32)
    for c in range(N_CORES):
        b, h = c // 2, c % 2
        out[b, h * SQ : (h + 1) * SQ, :] = res.results[c]["out"]
    if run_kwargs.get("trace"):
        kernel.last_results = res
    return out


# revision 13
# speedup vs baseline: 1.0101x; 1.0023x over previous
"""Full-width attention (B=4, S=2048, D=1024, no head split) on 8 TRN2 cores.

Sharding: data-parallel over (batch, query-half) -> 8 shards. Core c handles
batch b = c//2, query rows [h*1024, (h+1)*1024) with h = c%2.

v2 rewrite vs the K-side-folding baseline:
  - Fold Wq/Wk into the QUERY side: Q'' = (x_q M + w3) / 8 with M = Wq^T Wk
    and w3 = Wk^T bq. Then scores^T[k,q] = sum_e x[k,e] Q''[q,e] needs NO key
    projection at all -- the redundant per-pair M x^T (128 MMs) and the t3
    bias matmuls (32 MMs) disappear. The per-key softmax bias folds into Q''
    as a per-partition bias on the projection evacuation (free on Act).
  - All big matmuls run bf16 x bf16: same 1 cycle/row streaming as f32r but
    the 128-col weight load uses FWL (2x) and hides under the reorder
    window, cutting the per-MM gap. Also halves SBUF/DMA so x, x^T, E, Wv
    are all SBUF-resident -- zero DMA in the steady state.
  - Query half selection without a separate upload: the host rotates the
    KEY axis by h*1024 in both x^T (scores lhsT) and x (PX lhsT); attention
    is permutation-invariant over keys, and the core's queries are always
    columns 0:1024 of its rotated x^T.
  - Softmax without max-subtraction (|scores| <= ~25, exp safe in f32):
    E = exp(scores^T), rowsum via DVE accumulation + one ones-matmul
    partition-reduce, [1,1024] -> [128,8] recips via DRAM bounce.
  - out[q,e] = (x^T E)^T Wv^T scaled by 1/rowsum + bv (bv folded in after
    normalization since softmax rows sum to 1).
"""

import math
from contextlib import ExitStack

import numpy as np

P = 128
B, S, D = 4, 2048, 1024
SQ = 1024  # query rows per core
KO8 = 8  # 1024 contraction / 128
KO16 = 16  # 2048 contraction / 128
N_CORES = 8


def build_bass():
    from concourse import bacc
    import concourse.mybir as mybir
    from concourse.tile import TileContext

    f32 = mybir.dt.float32
    f32r = mybir.dt.float32r
    bf16 = mybir.dt.bfloat16
    AF = mybir.ActivationFunctionType

    nc = bacc.Bacc(
        "TRN2",
        target_bir_lowering=False,
        debug=False,
        enable_asserts=False,
        num_devices=N_CORES,
    )

    xT = nc.dram_tensor("xT", [D, S], bf16, kind="ExternalInput")
    xn = nc.dram_tensor("xn", [S, D], bf16, kind="ExternalInput")
    mT = nc.dram_tensor("mT", [D, D], bf16, kind="ExternalInput")
    wvT = nc.dram_tensor("wvT", [D, D], bf16, kind="ExternalInput")
    w3 = nc.dram_tensor("w3", [P, KO8], f32, kind="ExternalInput")
    bvb = nc.dram_tensor("bvb", [P, D], f32, kind="ExternalInput")
    out = nc.dram_tensor("out", [SQ, D], f32, kind="ExternalOutput")

    xT_r = xT[:, :].rearrange("(ko p) s -> p ko s", p=P)
    xn_r = xn[:, :].rearrange("(ko p) d -> p ko d", p=P)
    mT_r = mT[:, :].rearrange("(ko p) e -> p ko e", p=P)
    wvT_r = wvT[:, :].rearrange("(ko p) e -> p ko e", p=P)

    with TileContext(nc) as tc, ExitStack() as ctx:
        cst_p = ctx.enter_context(tc.tile_pool(name="cst", bufs=1))
        big_p = ctx.enter_context(tc.tile_pool(name="big", bufs=1))
        out_p = ctx.enter_context(tc.tile_pool(name="osp", bufs=3))
        psA_p = ctx.enter_context(tc.tile_pool(name="psA", bufs=3, space="PSUM"))
        psB_p = ctx.enter_context(tc.tile_pool(name="psB", bufs=2, space="PSUM"))
        psC_p = ctx.enter_context(tc.tile_pool(name="psC", bufs=2, space="PSUM"))
        psR_p = ctx.enter_context(tc.tile_pool(name="psR", bufs=1, space="PSUM"))
        dram_p = ctx.enter_context(tc.tile_pool(name="drp", bufs=1, space="DRAM"))

        # warmup operand comes from a memset, not a DMA, so the PE can start
        # ramping the instant the engines clear the startup barrier (walrus
        # rejects memset on f32r tiles, so memset f32 and bitcast for the PE)
        # no gpsimd instructions anywhere in this kernel: keeping the Q7
        # DSP out of the engine set shrinks the startup barrier / drain
        ones_f = cst_p.tile([P, 512], f32, tag="ones", name="ones_f")
        nc.vector.memset(ones_f[:], 1.0)
        ones_t = ones_f[:, :].bitcast(f32r)
        w3_t = cst_p.tile([P, KO8], f32, tag="w3", name="w3_t")
        bvb_t = cst_p.tile([P, D], f32, tag="bvb", name="bvb_t")

        # big residents
        xt_sb = big_p.tile([P, KO8, S], bf16, tag="xt", name="xt_sb")
        xn_sb = big_p.tile([P, KO16, D], bf16, tag="xn", name="xn_sb")
        m_sb = big_p.tile([P, KO8, D], bf16, tag="m", name="m_sb")
        wv_sb = big_p.tile([P, KO8, D], bf16, tag="wv", name="wv_sb")
        qt_sb = big_p.tile([P, KO8, SQ], bf16, tag="qt", name="qt_sb")
        e_sb = [
            big_p.tile([P, KO16, 512], bf16, tag=f"E{qc}", name=f"e_sb{qc}")
            for qc in range(2)
        ]
        px_sb = big_p.tile([P, KO8, SQ], bf16, tag="px", name="px_sb")
        racc = [
            cst_p.tile([P, 512], f32r, tag=f"racc{qc}", name=f"racc{qc}")
            for qc in range(2)
        ]
        rs_dram = dram_p.tile([1, SQ], f32, tag="rsd", name="rs_dram")

        # Phase-1 feed: per-ko full-width chunks (2KB/partition contiguous —
        # finer column splits halve DMA efficiency), m on the sync ring and
        # x^T query columns on the scalar ring in parallel. The first Q''
        # group then starts as soon as chunk ko=0 of both lands (~12.5us)
        # and pipelines with the remaining arrivals. Everything else is
        # emitted BEHIND phase-1 evacuations on the scalar queue so its
        # transfers cannot steal HBM bandwidth from the critical path.
        for ko in range(KO8):
            nc.sync.dma_start(m_sb[:, ko, :], mT_r[:, ko, :])
            nc.scalar.dma_start(xt_sb[:, ko, 0:SQ], xT_r[:, ko, 0:SQ])
        # small consts ride the scalar ring behind the critical chunks
        # (w3 first used at the first Q'' evacuation ~30us in)
        nc.scalar.dma_start(w3_t[:], w3[:, :])
        nc.scalar.dma_start(bvb_t[:], bvb[:, :])

        # PE warm-up on the ones tile: keeps the HAM activity window busy so
        # real matmuls run at 2.4 GHz, and covers the phase-1 DMA latency.
        warm = psR_p.tile([1, 512], f32, tag="psR", name="warm")
        for _ in range(14):
            nc.tensor.matmul(warm[:], ones_t[:, 0:1], ones_t[:, :])

        # ---- Phase 1: Q''T[e, q] = M^T x_q^T + w3 (scaled by 1/8 on host) --
        # ko-OUTER with all 8 eo accumulations held open across the full
        # PSUM bank set: each ko step needs only chunk ko of m/x^T, so the
        # whole sweep paces with the DMA chunk arrivals instead of
        # head-of-line blocking on the first eo group's last chunk.
        def q_sweep(qc):
            banks = [
                psA_p.tile([P, 512], f32, tag="psA", name=f"qp{qc}a{i}")
                for i in range(3)
            ] + [
                psB_p.tile([P, 512], f32, tag="psB", name=f"qp{qc}b{i}")
                for i in range(2)
            ] + [
                psC_p.tile([P, 512], f32, tag="psC", name=f"qp{qc}c{i}")
                for i in range(2)
            ] + [psR_p.tile([P, 512], f32, tag="psR", name=f"qp{qc}r")]
            for ko in range(KO8):
                for eo in range(KO8):
                    nc.tensor.matmul(
                        banks[eo][:],
                        m_sb[:, ko, eo * P : (eo + 1) * P],
                        xt_sb[:, ko, qc * 512 : (qc + 1) * 512],
                        start=(ko == 0), stop=(ko == KO8 - 1),
                    )
            for eo in range(KO8):
                nc.scalar.activation(
                    qt_sb[:, eo, qc * 512 : (qc + 1) * 512],
                    banks[eo][:], AF.Identity, bias=w3_t[:, eo : eo + 1],
                )

        q_sweep(0)
        # non-critical loads issue behind the first evacuations so their
        # transfers cannot steal HBM bandwidth from the phase-1 chunks
        for kp in range(2):
            nc.scalar.dma_start(
                xt_sb[:, 4 * kp : 4 * kp + 4, SQ:S],
                xT_r[:, 4 * kp : 4 * kp + 4, SQ:S],
            )
        nc.scalar.dma_start(wv_sb[:, :, :], wvT_r[:, :, :])
        q_sweep(1)
        for kp in range(2):
            nc.scalar.dma_start(
                xn_sb[:, 8 * kp : 8 * kp + 8, :], xn_r[:, 8 * kp : 8 * kp + 8, :]
            )

        # ---- Phase 2: scores^T -> exp -> E (bf16), rowsum acc on DVE ------
        for kidx in range(KO16):
            pa = psA_p.tile([P, 512], f32, tag="psA", name="spa")
            pb = psB_p.tile([P, 512], f32, tag="psB", name="spb")
            for eo in range(KO8):
                lh = xt_sb[:, eo, kidx * P : (kidx + 1) * P]
                nc.tensor.matmul(
                    pa[:], lh, qt_sb[:, eo, 0:512],
                    start=(eo == 0), stop=(eo == KO8 - 1),
                )
                nc.tensor.matmul(
                    pb[:], lh, qt_sb[:, eo, 512:1024],
                    start=(eo == 0), stop=(eo == KO8 - 1),
                )
            nc.scalar.activation(e_sb[0][:, kidx, :], pa[:], AF.Exp)
            nc.scalar.activation(e_sb[1][:, kidx, :], pb[:], AF.Exp)
            for qc in range(2):
                if kidx == 0:
                    nc.vector.tensor_copy(racc[qc][:], e_sb[qc][:, 0, :])
                else:
                    nc.vector.tensor_add(
                        racc[qc][:], racc[qc][:], e_sb[qc][:, kidx, :]
                    )

        # ---- Phase 3: PX^T[d, q] = sum_k x[k, d] E[k, q] -------------------
        for dc in range(KO8):
            pp = psA_p.tile([P, 512], f32, tag="psA", name="ppx")
            for ko in range(KO16):
                nc.tensor.matmul(
                    pp[:],
                    xn_sb[:, ko, dc * P : (dc + 1) * P],
                    e_sb[0][:, ko, :],
                    start=(ko == 0), stop=(ko == KO16 - 1),
                )
            nc.scalar.copy(px_sb[:, dc, 0:512], pp[:])

        # rowsum partition-reduce + [1,1024] -> [128,8] recip via DRAM bounce
        # (PE cost ~2 tiny matmuls; bounce hides under PX)
        for qc in range(2):
            pr = psR_p.tile([1, 512], f32, tag="psR", name="pr")
            nc.tensor.matmul(pr[:], ones_t[:, 0:1], racc[qc][:])
            rrow = cst_p.tile([1, 512], f32, tag=f"rr{qc}", name=f"rrow{qc}")
            nc.scalar.copy(rrow[:], pr[:])
            nc.sync.dma_start(rs_dram[0:1, qc * 512 : (qc + 1) * 512], rrow[:])
        rsum_t = cst_p.tile([P, 8], f32, tag="rst", name="rsum_t")
        nc.sync.dma_start(rsum_t[:, :], rs_dram[0, :].rearrange("(g p) -> p g", p=P))
        recip = cst_p.tile([P, 8], f32, tag="recip", name="recip")
        nc.vector.reciprocal(recip[:], rsum_t[:])

        for dc in range(KO8):
            pp = psA_p.tile([P, 512], f32, tag="psA", name="ppx")
            for ko in range(KO16):
                nc.tensor.matmul(
                    pp[:],
                    xn_sb[:, ko, dc * P : (dc + 1) * P],
                    e_sb[1][:, ko, :],
                    start=(ko == 0), stop=(ko == KO16 - 1),
                )
            nc.scalar.copy(px_sb[:, dc, 512:1024], pp[:])

        # ---- Phase 4: out[q, e] = PX^T.T Wv^T / rowsum + bv ---------------
        for g in range(8):
            pb = psB_p.tile([P, 512], f32, tag="psB", name="avb")
            pc = psC_p.tile([P, 512], f32, tag="psC", name="avc")
            for dc in range(KO8):
                lh = px_sb[:, dc, g * P : (g + 1) * P]
                nc.tensor.matmul(
                    pb[:], lh, wv_sb[:, dc, 0:512],
                    start=(dc == 0), stop=(dc == KO8 - 1),
                )
                nc.tensor.matmul(
                    pc[:], lh, wv_sb[:, dc, 512:1024],
                    start=(dc == 0), stop=(dc == KO8 - 1),
                )
            # fused (psum * recip) + bv straight from PSUM on DVE; each half
            # DMAs out as soon as its fuse lands so the tail never waits on
            # a combined transfer
            o = out_p.tile([P, D], f32, tag="ost", name="ost")
            mul, add = mybir.AluOpType.mult, mybir.AluOpType.add
            # (Pool/gpsimd cannot read PSUM on TRN2 — both halves on DVE)
            for half, ps, eng in ((0, pb, nc.vector), (1, pc, nc.vector)):
                eng.scalar_tensor_tensor(
                    o[:, half * 512 : (half + 1) * 512],
                    ps[:],
                    recip[:, g : g + 1],
                    bvb_t[:, half * 512 : (half + 1) * 512],
                    mul,
                    add,
                )
                nc.sync.dma_start(
                    out[g * P : (g + 1) * P, half * 512 : (half + 1) * 512],
                    o[:, half * 512 : (half + 1) * 512],
                )

    nc.finalize()
    return nc


def make_in_maps(x, Wq, bq, Wk, bk, Wv, bv):
    """Build the 8 per-core input maps from full inputs."""
    import ml_dtypes

    bf = ml_dtypes.bfloat16
    x = np.asarray(x, dtype=np.float32)
    inv8 = 1.0 / math.sqrt(D // 16)  # 1/sqrt(d_key=64) = 1/8
    # scores = x_q (Wq^T Wk) x_k^T / 8 + x_k.(Wk^T bq)/8 (+ softmax-invariant
    # per-query terms, dropped). Both folded into the query-side projection.
    M8 = (
        (np.asarray(Wq, np.float64).T @ np.asarray(Wk, np.float64)) * inv8
    ).astype(bf)
    w3 = (
        (np.asarray(Wk, np.float64).T @ np.asarray(bq, np.float64)) * inv8
    ).astype(np.float32)
    w3_np = np.ascontiguousarray(w3.reshape(KO8, P).T)
    wvT = np.ascontiguousarray(np.asarray(Wv, np.float32).T.astype(bf))
    bvb = np.ascontiguousarray(
        np.broadcast_to(np.asarray(bv, np.float32), (P, D))
    )
    in_maps = []
    for c in range(N_CORES):
        b, h = c // 2, c % 2
        # rotate the key axis by h*SQ so this core's queries are always
        # columns 0:SQ of xT; attention is permutation-invariant over keys
        # as long as xT (scores lhsT) and xn (PX lhsT) rotate together.
        xb = np.roll(x[b], -h * SQ, axis=0)
        in_maps.append(
            {
                "xT": np.ascontiguousarray(xb.T.astype(bf)),
                "xn": np.ascontiguousarray(xb.astype(bf)),
                "mT": M8,
                "wvT": wvT,
                "w3": w3_np,
                "bvb": bvb,
            }
        )
    return in_maps


_NC_CACHE = None


def get_nc():
    global _NC_CACHE
    if _NC_CACHE is None:
        _NC_CACHE = build_bass()
    return _NC_CACHE


def kernel(x, Wq, bq, Wk, bk, Wv, bv, **run_kwargs):
    from concourse.bass_utils import run_bass_kernel_spmd

    nc = get_nc()
    in_maps = make_in_maps(x, Wq, bq, Wk, bk, Wv, bv)
    res = run_bass_kernel_spmd(
        nc, in_maps, core_ids=list(range(N_CORES)), **run_kwargs
    )
    out = np.empty((B, S, D), dtype=np.float32)
    for c in range(N_CORES):
        b, h = c // 2, c % 2
        out[b, h * SQ : (h + 1) * SQ, :] = res.results[c]["out"]
    if run_kwargs.get("trace"):
        kernel.last_results = res
    return out
